# revision 27
# baseline (speedup 1.0000x reference)
"""Trainium2 Bass kernel for nn_MultiHeadedAttention_4054449128000.

Tensor-parallel over 8 NeuronCores (one TRN2 chip), bf16 matmuls:

  - Wq/Wk/Wv column-sharded (8 heads/core), all weights + activations
    pre-rounded to bf16 on host (halves HBM traffic vs fp32; fp32 PSUM
    accumulation keeps the contraction exact).
  - Projection order Q -> K -> V: the per-head cosine-similarity scores
    (which need only qh, kh) are computed on DVE while V's weights
    stream, and AllGathered (2 KB, fp32) as the FIRST collective. That
    small gather absorbs the NEFF entry barrier, the ~12us ncfw wake,
    and the cold-firmware penalty; the 67.6 KB vh AllGather right
    behind it then runs on warm firmware at the ~12us RDH floor.
    (The vh payload is padded past 64 KB: at 64 KB the runtime picks
    Mesh, which measures ~3x slower than RDH here.)
  - After the gathers every core computes exp(scores^T) for all 64
    heads, the cross-head einsum attn2[j, (d, b)] = exp(s)^T.T @ vh2
    (einsum half 1 lands in PSUM partitions 64-127, so softmax
    normalization is one full-width 128-partition DVE scale into the
    bf16 attn2 -- no partition-shift DMA), and its 512 output columns:
    out[:, cols_c] = attn @ Wo[:, cols_c] as 32 K=128 matmuls over
    host-packed, partition-major Wo streamed during the collectives.
  - Host concatenates the 8 (64, 512) output slices.
"""

import numpy as np

B = 64          # batch (== H, and j index of the cross-batch matmul)
L = 4096        # input feature dim
E = 4096        # projection dim
H = 64          # heads
DK = 64         # head dim
NC = 8          # cores
PC = E // NC    # 512 output columns per core
HL = H // NC    # 8 heads per core
KT = L // 128   # 32 k-tiles per projection
EPS = 1e-8
WCH = 4         # k-tiles per weight-chunk DMA
TT = B * DK // 128  # 32 stacked k-tiles of the final matmul

_CACHE = {}


def _build():
    import concourse.bacc as bacc
    import concourse.mybir as mybir
    import concourse.tile as tile
    from concourse.masks import make_identity

    f32 = mybir.dt.float32
    bf16 = mybir.dt.bfloat16
    AF = mybir.ActivationFunctionType
    ALU = mybir.AluOpType
    AX = mybir.AxisListType

    nc = bacc.Bacc("TRN2", target_bir_lowering=False, debug=False, num_devices=NC)

    xq = nc.declare_dram_parameter("xq", [128, KT * B], bf16, isOutput=False)
    xk = nc.declare_dram_parameter("xk", [128, KT * B], bf16, isOutput=False)
    xv = nc.declare_dram_parameter("xv", [128, KT * B], bf16, isOutput=False)
    # p-major packed weights: wq[p, kt*PC + n] = Wq[kt*128 + p, cols_c[n]]
    wq = nc.declare_dram_parameter("wq", [128, KT * PC], bf16, isOutput=False)
    wk = nc.declare_dram_parameter("wk", [128, KT * PC], bf16, isOutput=False)
    wv = nc.declare_dram_parameter("wv", [128, KT * PC], bf16, isOutput=False)
    # stacked p-major Wo: wo[p, t*PC + e] = Wo[(j=p%64)*DK + t + 32*(p//64), cols_c[e]]
    wo = nc.declare_dram_parameter("wo", [128, TT * PC], bf16, isOutput=False)
    bq = nc.declare_dram_parameter("bq", [1, PC], bf16, isOutput=False)
    bk = nc.declare_dram_parameter("bk", [1, PC], bf16, isOutput=False)
    bv = nc.declare_dram_parameter("bv", [1, PC], bf16, isOutput=False)
    bo = nc.declare_dram_parameter("bo", [1, PC], bf16, isOutput=False)
    ones = nc.declare_dram_parameter("ones", [1, B], bf16, isOutput=False)
    onescol = nc.declare_dram_parameter("onescol", [H, 1], bf16, isOutput=False)
    out = nc.declare_dram_parameter("out", [B, PC], f32, isOutput=True)

    with tile.TileContext(nc) as tc:
        with (
            tc.tile_pool(name="consts", bufs=1) as consts,
            tc.tile_pool(name="wpool", bufs=8) as wpool,
            tc.tile_pool(name="wopool", bufs=8) as wopool,
            tc.tile_pool(name="work", bufs=1) as work,
            tc.tile_pool(name="pp", bufs=3, space="PSUM") as pp,
            tc.tile_pool(name="pa", bufs=1, space="PSUM") as pa,
            tc.tile_pool(name="po", bufs=1, space="PSUM") as po,
            tc.tile_pool(name="dram", bufs=1, space="DRAM") as dram,
        ):
            # ---- constants / activations in ----
            xt = {}
            xsrc = {"v": xv, "q": xq, "k": xk}

            def load_x(name):
                t = consts.tile([128, KT * B], bf16, tag=f"x{name}")
                nc.sync.dma_start(t[:], xsrc[name][:])
                xt[name] = t

            load_x("q")
            ones_t = consts.tile([1, B], bf16, tag="ones")
            nc.sync.dma_start(ones_t[:], ones[:])
            onescol_t = consts.tile([H, 1], bf16, tag="onescol")
            nc.sync.dma_start(onescol_t[:], onescol[:])
            bias_t = {}
            for name, src in (("q", bq), ("k", bk), ("v", bv), ("o", bo)):
                t = consts.tile([1, PC], bf16, tag=f"b{name}")
                nc.sync.dma_start(t[:], src[:])
                bias_t[name] = t
            ident = consts.tile([B, B], f32, tag="ident")
            make_identity(nc, ident)

            # Warm the Exp/Sqrt activation tables off the critical path.
            warm = work.tile([1, 1], f32, tag="warm")
            nc.gpsimd.memset(warm[:], 0.0)
            nc.scalar.activation(warm[:], warm[:], AF.Exp)
            nc.scalar.activation(warm[:], warm[:], AF.Sqrt)

            ag1_in = dram.tile([HL, B], f32)                 # scores^T shard
            ag1_out = dram.tile([NC, HL, B], f32, addr_space="Shared")
            # vh payload padded past 64 KB so the runtime picks RDH, not
            # the much slower Mesh algorithm at this size
            ag2_in = dram.tile([PC + 16, B], bf16)
            ag2_out = dram.tile([NC, PC + 16, B], bf16, addr_space="Shared")

            # ---- projections (bf16, K tiled by 128); Q, K first, V last ----
            CHUNKS = [1, 1, 2] + [4] * 7

            def project(name, w, bias, after_chunk=None):
                ps = pp.tile([B, PC], f32, tag="pp")
                kt0 = 0
                for ci, chn in enumerate(CHUNKS):
                    wt = wpool.tile([128, chn * PC], bf16, tag="w")
                    nc.sync.dma_start(wt[:], w[:, kt0 * PC:(kt0 + chn) * PC])
                    if after_chunk is not None and ci == 3:
                        after_chunk()
                    for i in range(chn):
                        kt = kt0 + i
                        nc.tensor.matmul(
                            ps[:],
                            lhsT=xt[name][:, kt * B:(kt + 1) * B],
                            rhs=wt[:, i * PC:(i + 1) * PC],
                            start=(kt == 0),
                            stop=False,
                        )
                    kt0 += chn
                nc.tensor.matmul(
                    ps[:], lhsT=ones_t[:], rhs=bias_t[bias][:], start=False, stop=True
                )
                return ps

            def _prefetch_xkv():
                load_x("k")
                load_x("v")

            qh_ps = project("q", wq, "q", after_chunk=_prefetch_xkv)
            qh_s = work.tile([B, PC], f32, tag="qh_s")
            nc.vector.tensor_copy(qh_s[:], qh_ps[:])
            kh_ps = project("k", wk, "k")
            kh_s = work.tile([B, PC], f32, tag="kh_s")
            nc.scalar.activation(kh_s[:], kh_ps[:], AF.Copy)

            # ---- local cosine-sim scores (64, 8); overlaps V's DMA ----
            prod = work.tile([B, PC], f32, tag="prod")
            red = work.tile([B, 3 * HL], f32, tag="red")  # num | qq | kk
            for i, (a, b) in enumerate(((qh_s, kh_s), (qh_s, qh_s), (kh_s, kh_s))):
                nc.vector.tensor_tensor(prod[:], a[:], b[:], op=ALU.mult)
                nc.vector.reduce_sum(
                    red[:, i * HL:(i + 1) * HL].unsqueeze(-1),
                    prod[:].rearrange("p (h d) -> p h d", d=DK),
                    axis=AX.X,
                )
            den = work.tile([B, HL], f32, tag="den")
            nc.vector.tensor_tensor(
                den[:], red[:, HL:2 * HL], red[:, 2 * HL:3 * HL], op=ALU.mult
            )
            nc.scalar.activation(den[:], den[:], AF.Sqrt)
            nc.vector.tensor_scalar_max(den[:], den[:], EPS)
            nc.vector.reciprocal(den[:], den[:])
            sc = work.tile([B, HL], f32, tag="sc")
            nc.vector.tensor_tensor(sc[:], red[:, 0:HL], den[:], op=ALU.mult)
            # transpose scores (64, 8) -> (8, 64); first (cold) AllGather
            scT_ps = pp.tile([HL, B], f32, tag="pp")
            nc.tensor.transpose(scT_ps[:], sc[:], ident[:])
            scT_s = work.tile([HL, B], f32, tag="scT_s")
            nc.vector.tensor_copy(scT_s[:], scT_ps[:])
            nc.scalar.dma_start(ag1_in[:], scT_s[:])
            nc.gpsimd.collective_compute(
                "AllGather",
                ALU.bypass,
                replica_groups=[list(range(NC))],
                ins=[ag1_in.opt()],
                outs=[ag1_out.opt()],
            )

            vh_ps = project("v", wv, "v")
            vh_s = work.tile([B, PC], f32, tag="vh_s")
            nc.vector.tensor_copy(vh_s[:], vh_ps[:])
            # transpose vh (64, 512) -> vhT (512, 64) in 4 PE transposes
            vhT_ps = pp.tile([128, 4 * B], f32, tag="pp")
            for t in range(4):
                nc.tensor.transpose(
                    vhT_ps[:, t * B:(t + 1) * B], vh_s[:, t * 128:(t + 1) * 128], ident[:]
                )
            vhT_s = work.tile([128, 4 * B], bf16, tag="vhT_s")
            nc.scalar.activation(vhT_s[:], vhT_ps[:], AF.Copy)
            nc.sync.dma_start(
                ag2_in[0:PC, :].rearrange("(t p) b -> p t b", p=128),
                vhT_s[:].rearrange("p (t b) -> p t b", t=4),
            )
            # fill the 2 KB pad region (its values are never read)
            nc.scalar.dma_start(
                ag2_in[PC:PC + 16, :].bitcast(f32).rearrange("(a hl) b -> a hl b", a=2),
                scT_s[:].rearrange("hl (a b) -> a hl b", a=2),
            )
            # second (warm) AllGather: the vh head-shard
            nc.gpsimd.collective_compute(
                "AllGather",
                ALU.bypass,
                replica_groups=[list(range(NC))],
                ins=[ag2_in.opt()],
                outs=[ag2_out.opt()],
            )

            # ---- Wo chunks stream while the collectives run ----
            wo_t = []
            for ch in range(TT // WCH):
                t = wopool.tile([128, WCH * PC], bf16, tag="wo")
                nc.sync.dma_start(t[:], wo[:, ch * WCH * PC:(ch + 1) * WCH * PC])
                wo_t.append(t)

            # ---- softmax pieces: sT, exp, row sums (after AG1) ----
            sT = work.tile([H, B], f32, tag="sT")
            nc.scalar.dma_start(sT[:], ag1_out[:].rearrange("c hl b -> (c hl) b"))
            expT = work.tile([H, B], bf16, tag="expT")
            nc.scalar.activation(expT[:], sT[:], AF.Exp)
            rT_ps = pp.tile([1, B], f32, tag="pp")
            nc.tensor.matmul(rT_ps[:], lhsT=onescol_t[:], rhs=expT[:], start=True, stop=True)
            rT2 = work.tile([1, 2 * B], f32, tag="rT2")
            nc.vector.reciprocal(rT2[:, 0:B], rT_ps[:])
            nc.vector.reciprocal(rT2[:, B:2 * B], rT_ps[:])
            rinv_ps = pp.tile([128, 1], f32, tag="pp")
            nc.tensor.transpose(rinv_ps[:], rT2[:], ident[0:1, 0:1])
            rinv = work.tile([128, 1], f32, tag="rinv")
            nc.vector.tensor_copy(rinv[:], rinv_ps[:])

            # vh2[h, (d, b)] for all 64 heads, spread across 4 engine queues
            vh2 = work.tile([H, DK * B], bf16, tag="vh2")
            qeng = [nc.sync, nc.scalar]
            for c in range(NC):
                qeng[c % 2].dma_start(
                    vh2[c * HL:(c + 1) * HL].rearrange("hl (d b) -> hl d b", d=DK),
                    ag2_out[c, 0:PC, :].rearrange("(hl d) b -> hl d b", hl=HL),
                )

            # PE p-state warm-up: the PE sat idle through the collectives
            # and restarts at 1.2 GHz; a short chain of throwaway matmuls
            # gated only on vh2's first slot ramps it back to 2.4 GHz just
            # before the einsum + final matmuls.
            pwarm = pp.tile([B, PC], f32, tag="pp")
            for w in range(8):
                nc.tensor.matmul(
                    pwarm[:],
                    lhsT=expT[0:HL, :],
                    rhs=vh2[0:HL, 0:PC],
                    start=True,
                    stop=(w == 7),
                )

            # ---- einsum attn2[j, (d, b)] = expT.T @ vh2; half 1 lands in
            #      PSUM partitions 64-127, so normalization is one
            #      full-width DVE scale into bf16 attn2 ----
            HALF = DK * B // 2
            attn2 = work.tile([128, TT * B], bf16, tag="attn2")
            pah = pa.tile([128, HALF], f32, tag="pa")
            for h in range(2):
                for n in range(HALF // 512):
                    nc.tensor.matmul(
                        pah[h * B:(h + 1) * B, n * 512:(n + 1) * 512],
                        lhsT=expT[:],
                        rhs=vh2[:, h * HALF + n * 512:h * HALF + (n + 1) * 512],
                        start=True,
                        stop=True,
                    )
            nc.vector.tensor_scalar_mul(attn2[:, :], pah[:, :], rinv[:])

            # ---- out[:, cols_c] = attn @ Wo_c + bo_c : 32 K=128 matmuls ----
            op = po.tile([B, PC], f32, tag="po")
            for t in range(TT):
                nc.tensor.matmul(
                    op[:],
                    lhsT=attn2[:, t * B:(t + 1) * B],
                    rhs=wo_t[t // WCH][:, (t % WCH) * PC:(t % WCH + 1) * PC],
                    start=(t == 0),
                    stop=False,
                )
            nc.tensor.matmul(
                op[:], lhsT=ones_t[:], rhs=bias_t["o"][:], start=False, stop=True
            )
            out_s = work.tile([B, PC], f32, tag="out_s")
            nc.vector.tensor_copy(out_s[:], op[:])
            nc.sync.dma_start(out[:], out_s[:])

    nc.compile()
    return nc


def _prep_inputs(q, k, v, Wq, bq, Wk, bk, Wv, bv, Wo, bo):
    import ml_dtypes

    bf16 = ml_dtypes.bfloat16

    def xt_layout(x):
        a = np.ascontiguousarray(x[:, 0, :].T)          # (L, B)
        a = a.reshape(KT, 128, B).transpose(1, 0, 2)     # (128, KT, B)
        return np.ascontiguousarray(a).reshape(128, KT * B).astype(bf16)

    def w_layout(W, cols):
        a = W[:, cols].reshape(KT, 128, PC).transpose(1, 0, 2)  # (128, KT, PC)
        return np.ascontiguousarray(a).reshape(128, KT * PC).astype(bf16)

    def wo_layout(Wo_c):
        # wo[p, t*PC + e] = Wo_c[(p % 64) * DK + t + 32 * (p // 64), e]
        a = Wo_c.reshape(B, 2, TT, PC).transpose(1, 0, 2, 3)    # (hi, j, t, e)
        return np.ascontiguousarray(a).reshape(128, TT * PC).astype(bf16)

    xq_np, xk_np, xv_np = xt_layout(q), xt_layout(k), xt_layout(v)
    ones_np = np.ones((1, B), dtype=bf16)

    in_maps = []
    for c in range(NC):
        cols = slice(c * PC, (c + 1) * PC)
        in_maps.append({
            "xq": xq_np, "xk": xk_np, "xv": xv_np,
            "wq": w_layout(Wq, cols),
            "wk": w_layout(Wk, cols),
            "wv": w_layout(Wv, cols),
            "wo": wo_layout(np.ascontiguousarray(Wo[:, cols])),
            "bq": bq[cols].reshape(1, PC).astype(bf16),
            "bk": bk[cols].reshape(1, PC).astype(bf16),
            "bv": bv[cols].reshape(1, PC).astype(bf16),
            "bo": bo[cols].reshape(1, PC).astype(bf16),
            "ones": ones_np,
            "onescol": np.ones((H, 1), dtype=bf16),
        })
    return in_maps


def _run(inputs, trace=False, **kw):
    from concourse.bass_utils import run_bass_kernel_spmd

    if "nc" not in _CACHE:
        _CACHE["nc"] = _build()
    nc = _CACHE["nc"]
    in_maps = _prep_inputs(**inputs)
    res = run_bass_kernel_spmd(nc, in_maps, list(range(NC)), trace=trace, **kw)
    full = np.concatenate([res.results[c]["out"] for c in range(NC)], axis=1)
    return full.astype(np.float32), res


def kernel(**inputs):
    out, _ = _run(inputs, trace=False)
    return out


# revision 28
# speedup vs baseline: 1.0166x; 1.0166x over previous
"""Trainium2 Bass kernel for nn_MultiHeadedAttention_4054449128000.

Tensor-parallel over 8 NeuronCores (one TRN2 chip), bf16 matmuls:

  - Wq/Wk/Wv column-sharded (8 heads/core), all weights + activations
    pre-rounded to bf16 on host (halves HBM traffic vs fp32; fp32 PSUM
    accumulation keeps the contraction exact).
  - Projection order Q -> K -> V: the per-head cosine-similarity scores
    (which need only qh, kh) are computed on DVE while V's weights
    stream, and AllGathered (2 KB, fp32) as the FIRST collective. That
    small gather absorbs the NEFF entry barrier, the ~12us ncfw wake,
    and the cold-firmware penalty; the 67.6 KB vh AllGather right
    behind it then runs on warm firmware at the ~12us RDH floor.
    (The vh payload is padded past 64 KB: at 64 KB the runtime picks
    Mesh, which measures ~3x slower than RDH here.)
  - After the gathers every core computes exp(scores^T) for all 64
    heads, the cross-head einsum attn2[j, (d, b)] = exp(s)^T.T @ vh2
    (einsum half 1 lands in PSUM partitions 64-127, so softmax
    normalization is one full-width 128-partition DVE scale into the
    bf16 attn2 -- no partition-shift DMA), and its 512 output columns:
    out[:, cols_c] = attn @ Wo[:, cols_c] as 32 K=128 matmuls over
    host-packed, partition-major Wo streamed during the collectives.
  - Host concatenates the 8 (64, 512) output slices.
"""

import numpy as np

B = 64          # batch (== H, and j index of the cross-batch matmul)
L = 4096        # input feature dim
E = 4096        # projection dim
H = 64          # heads
DK = 64         # head dim
NC = 8          # cores
PC = E // NC    # 512 output columns per core
HL = H // NC    # 8 heads per core
KT = L // 128   # 32 k-tiles per projection
EPS = 1e-8
WCH = 4         # k-tiles per weight-chunk DMA
TT = B * DK // 128  # 32 stacked k-tiles of the final matmul

_CACHE = {}


def _build():
    import concourse.bacc as bacc
    import concourse.mybir as mybir
    import concourse.tile as tile
    from concourse.masks import make_identity

    f32 = mybir.dt.float32
    bf16 = mybir.dt.bfloat16
    AF = mybir.ActivationFunctionType
    ALU = mybir.AluOpType
    AX = mybir.AxisListType

    nc = bacc.Bacc("TRN2", target_bir_lowering=False, debug=False, num_devices=NC)

    xq = nc.declare_dram_parameter("xq", [128, KT * B], bf16, isOutput=False)
    xk = nc.declare_dram_parameter("xk", [128, KT * B], bf16, isOutput=False)
    xv = nc.declare_dram_parameter("xv", [128, KT * B], bf16, isOutput=False)
    # p-major packed weights: wq[p, kt*PC + n] = Wq[kt*128 + p, cols_c[n]]
    wq = nc.declare_dram_parameter("wq", [128, KT * PC], bf16, isOutput=False)
    wk = nc.declare_dram_parameter("wk", [128, KT * PC], bf16, isOutput=False)
    wv = nc.declare_dram_parameter("wv", [128, KT * PC], bf16, isOutput=False)
    # stacked p-major Wo: wo[p, t*PC + e] = Wo[(j=p%64)*DK + t + 32*(p//64), cols_c[e]]
    wo = nc.declare_dram_parameter("wo", [128, TT * PC], bf16, isOutput=False)
    bq = nc.declare_dram_parameter("bq", [1, PC], bf16, isOutput=False)
    bk = nc.declare_dram_parameter("bk", [1, PC], bf16, isOutput=False)
    bv = nc.declare_dram_parameter("bv", [1, PC], bf16, isOutput=False)
    bo = nc.declare_dram_parameter("bo", [1, PC], bf16, isOutput=False)
    ones = nc.declare_dram_parameter("ones", [1, B], bf16, isOutput=False)
    onescol = nc.declare_dram_parameter("onescol", [H, 1], bf16, isOutput=False)
    out = nc.declare_dram_parameter("out", [B, PC], f32, isOutput=True)

    with tile.TileContext(nc) as tc:
        with (
            tc.tile_pool(name="consts", bufs=1) as consts,
            tc.tile_pool(name="wpool", bufs=8) as wpool,
            tc.tile_pool(name="wopool", bufs=8) as wopool,
            tc.tile_pool(name="work", bufs=1) as work,
            tc.tile_pool(name="pp", bufs=3, space="PSUM") as pp,
            tc.tile_pool(name="pa", bufs=1, space="PSUM") as pa,
            tc.tile_pool(name="po", bufs=1, space="PSUM") as po,
            tc.tile_pool(name="dram", bufs=1, space="DRAM") as dram,
        ):
            # ---- constants / activations in ----
            xt = {}
            xsrc = {"v": xv, "q": xq, "k": xk}

            def load_x(name):
                t = consts.tile([128, KT * B], bf16, tag=f"x{name}")
                nc.sync.dma_start(t[:], xsrc[name][:])
                xt[name] = t

            load_x("q")
            ones_t = consts.tile([1, B], bf16, tag="ones")
            nc.sync.dma_start(ones_t[:], ones[:])
            onescol_t = consts.tile([H, 1], bf16, tag="onescol")
            nc.sync.dma_start(onescol_t[:], onescol[:])
            bias_t = {}
            for name, src in (("q", bq), ("k", bk), ("v", bv), ("o", bo)):
                t = consts.tile([1, PC], bf16, tag=f"b{name}")
                nc.sync.dma_start(t[:], src[:])
                bias_t[name] = t
            ident = consts.tile([B, B], f32, tag="ident")
            make_identity(nc, ident)

            # Warm the Exp/Sqrt activation tables off the critical path.
            warm = work.tile([1, 1], f32, tag="warm")
            nc.gpsimd.memset(warm[:], 0.0)
            nc.scalar.activation(warm[:], warm[:], AF.Exp)
            nc.scalar.activation(warm[:], warm[:], AF.Sqrt)

            ag1_in = dram.tile([HL, B], f32)                 # scores^T shard
            ag1_out = dram.tile([NC, HL, B], f32, addr_space="Shared")
            # vh payload padded past 64 KB so the runtime picks RDH, not
            # the much slower Mesh algorithm at this size
            ag2_in = dram.tile([PC + 16, B], bf16)
            ag2_out = dram.tile([NC, PC + 16, B], bf16, addr_space="Shared")

            # ---- projections (bf16, K tiled by 128); Q, K first, V last ----
            CHUNKS = [1, 1, 2] + [4] * 7

            def project(name, w, bias, after_chunk=None):
                ps = pp.tile([B, PC], f32, tag="pp")
                kt0 = 0
                for ci, chn in enumerate(CHUNKS):
                    wt = wpool.tile([128, chn * PC], bf16, tag="w")
                    nc.sync.dma_start(wt[:], w[:, kt0 * PC:(kt0 + chn) * PC])
                    if after_chunk is not None and ci == 3:
                        after_chunk()
                    for i in range(chn):
                        kt = kt0 + i
                        nc.tensor.matmul(
                            ps[:],
                            lhsT=xt[name][:, kt * B:(kt + 1) * B],
                            rhs=wt[:, i * PC:(i + 1) * PC],
                            start=(kt == 0),
                            stop=False,
                        )
                    kt0 += chn
                nc.tensor.matmul(
                    ps[:], lhsT=ones_t[:], rhs=bias_t[bias][:], start=False, stop=True
                )
                return ps

            def _prefetch_xkv():
                load_x("k")
                load_x("v")

            qh_ps = project("q", wq, "q", after_chunk=_prefetch_xkv)
            qh_s = work.tile([B, PC], f32, tag="qh_s")
            nc.vector.tensor_copy(qh_s[:], qh_ps[:])
            kh_ps = project("k", wk, "k")
            kh_s = work.tile([B, PC], f32, tag="kh_s")
            nc.scalar.activation(kh_s[:], kh_ps[:], AF.Copy)

            # ---- local cosine-sim scores (64, 8); overlaps V's DMA ----
            prod = work.tile([B, PC], f32, tag="prod")
            red = work.tile([B, 3 * HL], f32, tag="red")  # num | qq | kk
            for i, (a, b) in enumerate(((qh_s, kh_s), (qh_s, qh_s), (kh_s, kh_s))):
                nc.vector.tensor_tensor(prod[:], a[:], b[:], op=ALU.mult)
                nc.vector.reduce_sum(
                    red[:, i * HL:(i + 1) * HL].unsqueeze(-1),
                    prod[:].rearrange("p (h d) -> p h d", d=DK),
                    axis=AX.X,
                )
            den = work.tile([B, HL], f32, tag="den")
            nc.vector.tensor_tensor(
                den[:], red[:, HL:2 * HL], red[:, 2 * HL:3 * HL], op=ALU.mult
            )
            nc.scalar.activation(den[:], den[:], AF.Sqrt)
            nc.vector.tensor_scalar_max(den[:], den[:], EPS)
            nc.vector.reciprocal(den[:], den[:])
            sc = work.tile([B, HL], f32, tag="sc")
            nc.vector.tensor_tensor(sc[:], red[:, 0:HL], den[:], op=ALU.mult)
            # transpose scores (64, 8) -> (8, 64); first (cold) AllGather
            scT_ps = pp.tile([HL, B], f32, tag="pp")
            nc.tensor.transpose(scT_ps[:], sc[:], ident[:])
            scT_s = work.tile([HL, B], f32, tag="scT_s")
            nc.vector.tensor_copy(scT_s[:], scT_ps[:])
            nc.scalar.dma_start(ag1_in[:], scT_s[:])
            nc.gpsimd.collective_compute(
                "AllGather",
                ALU.bypass,
                replica_groups=[list(range(NC))],
                ins=[ag1_in.opt()],
                outs=[ag1_out.opt()],
            )

            vh_ps = project("v", wv, "v")
            vh_s = work.tile([B, PC], f32, tag="vh_s")
            nc.vector.tensor_copy(vh_s[:], vh_ps[:])
            # transpose vh (64, 512) -> vhT (512, 64) in 4 PE transposes
            vhT_ps = pp.tile([128, 4 * B], f32, tag="pp")
            for t in range(4):
                nc.tensor.transpose(
                    vhT_ps[:, t * B:(t + 1) * B], vh_s[:, t * 128:(t + 1) * 128], ident[:]
                )
            vhT_s = work.tile([128, 4 * B], bf16, tag="vhT_s")
            nc.scalar.activation(vhT_s[:], vhT_ps[:], AF.Copy)
            nc.sync.dma_start(
                ag2_in[0:PC, :].rearrange("(t p) b -> p t b", p=128),
                vhT_s[:].rearrange("p (t b) -> p t b", t=4),
            )
            # fill the 2 KB pad region (its values are never read)
            nc.scalar.dma_start(
                ag2_in[PC:PC + 16, :].bitcast(f32).rearrange("(a hl) b -> a hl b", a=2),
                scT_s[:].rearrange("hl (a b) -> a hl b", a=2),
            )
            # second (warm) AllGather: the vh head-shard
            nc.gpsimd.collective_compute(
                "AllGather",
                ALU.bypass,
                replica_groups=[list(range(NC))],
                ins=[ag2_in.opt()],
                outs=[ag2_out.opt()],
            )

            # ---- Wo chunks stream while the collectives run ----
            wo_t = []
            for ch in range(TT // WCH):
                t = wopool.tile([128, WCH * PC], bf16, tag="wo")
                nc.sync.dma_start(t[:], wo[:, ch * WCH * PC:(ch + 1) * WCH * PC])
                wo_t.append(t)

            # ---- softmax pieces: sT, exp, row sums (after AG1) ----
            sT = work.tile([H, B], f32, tag="sT")
            nc.scalar.dma_start(sT[:], ag1_out[:].rearrange("c hl b -> (c hl) b"))
            expT = work.tile([H, B], bf16, tag="expT")
            nc.scalar.activation(expT[:], sT[:], AF.Exp)
            rT_ps = pp.tile([1, B], f32, tag="pp")
            nc.tensor.matmul(rT_ps[:], lhsT=onescol_t[:], rhs=expT[:], start=True, stop=True)
            rT2 = work.tile([1, 2 * B], f32, tag="rT2")
            nc.vector.reciprocal(rT2[:, 0:B], rT_ps[:])
            nc.vector.reciprocal(rT2[:, B:2 * B], rT_ps[:])
            rinv_ps = pp.tile([128, 1], f32, tag="pp")
            nc.tensor.transpose(rinv_ps[:], rT2[:], ident[0:1, 0:1])
            rinv = work.tile([128, 1], f32, tag="rinv")
            nc.vector.tensor_copy(rinv[:], rinv_ps[:])

            # vh2[h, (d, b)] for all 64 heads, spread across 4 engine queues
            vh2 = work.tile([H, DK * B], bf16, tag="vh2")
            qeng = [nc.sync, nc.scalar]
            for c in range(NC):
                qeng[c % 2].dma_start(
                    vh2[c * HL:(c + 1) * HL].rearrange("hl (d b) -> hl d b", d=DK),
                    ag2_out[c, 0:PC, :].rearrange("(hl d) b -> hl d b", hl=HL),
                )

            # PE p-state warm-up: the PE sat idle through the collectives
            # and restarts at 1.2 GHz (~630ns per 512-row matmul instead of
            # ~215ns hot). Ramp needs several us of continuous execution,
            # so run throwaway matmuls on operands that become ready during
            # the collective window (expT after AG1, wo chunk 0 long
            # before), then a short slot-0-gated chain to bridge to vh2.
            pwarm = pp.tile([B, PC], f32, tag="pp")
            for w in range(20):
                nc.tensor.matmul(
                    pwarm[:],
                    lhsT=expT[:],
                    rhs=wo_t[0][0:H, 0:PC],
                    start=True,
                    stop=(w == 19),
                )
            for w in range(4):
                nc.tensor.matmul(
                    pwarm[:],
                    lhsT=expT[0:HL, :],
                    rhs=vh2[0:HL, 0:PC],
                    start=True,
                    stop=(w == 3),
                )

            # ---- einsum attn2[j, (d, b)] = expT.T @ vh2; half 1 lands in
            #      PSUM partitions 64-127, so normalization is one
            #      full-width DVE scale into bf16 attn2 ----
            HALF = DK * B // 2
            attn2 = work.tile([128, TT * B], bf16, tag="attn2")
            pah = pa.tile([128, HALF], f32, tag="pa")
            for h in range(2):
                for n in range(HALF // 512):
                    nc.tensor.matmul(
                        pah[h * B:(h + 1) * B, n * 512:(n + 1) * 512],
                        lhsT=expT[:],
                        rhs=vh2[:, h * HALF + n * 512:h * HALF + (n + 1) * 512],
                        start=True,
                        stop=True,
                    )
            nc.vector.tensor_scalar_mul(attn2[:, :], pah[:, :], rinv[:])

            # ---- out[:, cols_c] = attn @ Wo_c + bo_c : 32 K=128 matmuls ----
            op = po.tile([B, PC], f32, tag="po")
            for t in range(TT):
                nc.tensor.matmul(
                    op[:],
                    lhsT=attn2[:, t * B:(t + 1) * B],
                    rhs=wo_t[t // WCH][:, (t % WCH) * PC:(t % WCH + 1) * PC],
                    start=(t == 0),
                    stop=False,
                )
            nc.tensor.matmul(
                op[:], lhsT=ones_t[:], rhs=bias_t["o"][:], start=False, stop=True
            )
            out_s = work.tile([B, PC], f32, tag="out_s")
            nc.vector.tensor_copy(out_s[:], op[:])
            nc.sync.dma_start(out[:], out_s[:])

    nc.compile()
    return nc


def _prep_inputs(q, k, v, Wq, bq, Wk, bk, Wv, bv, Wo, bo):
    import ml_dtypes

    bf16 = ml_dtypes.bfloat16

    def xt_layout(x):
        a = np.ascontiguousarray(x[:, 0, :].T)          # (L, B)
        a = a.reshape(KT, 128, B).transpose(1, 0, 2)     # (128, KT, B)
        return np.ascontiguousarray(a).reshape(128, KT * B).astype(bf16)

    def w_layout(W, cols):
        a = W[:, cols].reshape(KT, 128, PC).transpose(1, 0, 2)  # (128, KT, PC)
        return np.ascontiguousarray(a).reshape(128, KT * PC).astype(bf16)

    def wo_layout(Wo_c):
        # wo[p, t*PC + e] = Wo_c[(p % 64) * DK + t + 32 * (p // 64), e]
        a = Wo_c.reshape(B, 2, TT, PC).transpose(1, 0, 2, 3)    # (hi, j, t, e)
        return np.ascontiguousarray(a).reshape(128, TT * PC).astype(bf16)

    xq_np, xk_np, xv_np = xt_layout(q), xt_layout(k), xt_layout(v)
    ones_np = np.ones((1, B), dtype=bf16)

    in_maps = []
    for c in range(NC):
        cols = slice(c * PC, (c + 1) * PC)
        in_maps.append({
            "xq": xq_np, "xk": xk_np, "xv": xv_np,
            "wq": w_layout(Wq, cols),
            "wk": w_layout(Wk, cols),
            "wv": w_layout(Wv, cols),
            "wo": wo_layout(np.ascontiguousarray(Wo[:, cols])),
            "bq": bq[cols].reshape(1, PC).astype(bf16),
            "bk": bk[cols].reshape(1, PC).astype(bf16),
            "bv": bv[cols].reshape(1, PC).astype(bf16),
            "bo": bo[cols].reshape(1, PC).astype(bf16),
            "ones": ones_np,
            "onescol": np.ones((H, 1), dtype=bf16),
        })
    return in_maps


def _run(inputs, trace=False, **kw):
    from concourse.bass_utils import run_bass_kernel_spmd

    if "nc" not in _CACHE:
        _CACHE["nc"] = _build()
    nc = _CACHE["nc"]
    in_maps = _prep_inputs(**inputs)
    res = run_bass_kernel_spmd(nc, in_maps, list(range(NC)), trace=trace, **kw)
    full = np.concatenate([res.results[c]["out"] for c in range(NC)], axis=1)
    return full.astype(np.float32), res


def kernel(**inputs):
    out, _ = _run(inputs, trace=False)
    return out


# revision 29
# speedup vs baseline: 1.0379x; 1.0209x over previous
"""Trainium2 Bass kernel for nn_MultiHeadedAttention_4054449128000.

Tensor-parallel over 8 NeuronCores (one TRN2 chip), bf16 matmuls:

  - Wq/Wk/Wv column-sharded (8 heads/core), all weights + activations
    pre-rounded to bf16 on host (halves HBM traffic vs fp32; fp32 PSUM
    accumulation keeps the contraction exact).
  - Projection order Q -> K -> V: the per-head cosine-similarity scores
    (which need only qh, kh) are computed on DVE while V's weights
    stream, and AllGathered (2 KB, fp32) as the FIRST collective. That
    small gather absorbs the NEFF entry barrier, the ~12us ncfw wake,
    and the cold-firmware penalty; the 67.6 KB vh AllGather right
    behind it then runs on warm firmware at the ~12us RDH floor.
    (The vh payload is padded past 64 KB: at 64 KB the runtime picks
    Mesh, which measures ~3x slower than RDH here.)
  - After the gathers every core computes exp(scores^T) for all 64
    heads, the cross-head einsum attn2[j, (d, b)] = exp(s)^T.T @ vh2
    (einsum half 1 lands in PSUM partitions 64-127, so softmax
    normalization is one full-width 128-partition DVE scale into the
    bf16 attn2 -- no partition-shift DMA), and its 512 output columns:
    out[:, cols_c] = attn @ Wo[:, cols_c] as 32 K=128 matmuls over
    host-packed, partition-major Wo streamed during the collectives.
  - Host concatenates the 8 (64, 512) output slices.
"""

import numpy as np

B = 64          # batch (== H, and j index of the cross-batch matmul)
L = 4096        # input feature dim
E = 4096        # projection dim
H = 64          # heads
DK = 64         # head dim
NC = 8          # cores
PC = E // NC    # 512 output columns per core
HL = H // NC    # 8 heads per core
KT = L // 128   # 32 k-tiles per projection
EPS = 1e-8
WCH = 4         # k-tiles per weight-chunk DMA
TT = B * DK // 128  # 32 stacked k-tiles of the final matmul

_CACHE = {}


def _build():
    import concourse.bacc as bacc
    import concourse.mybir as mybir
    import concourse.tile as tile
    from concourse.masks import make_identity

    f32 = mybir.dt.float32
    bf16 = mybir.dt.bfloat16
    AF = mybir.ActivationFunctionType
    ALU = mybir.AluOpType
    AX = mybir.AxisListType

    nc = bacc.Bacc("TRN2", target_bir_lowering=False, debug=False, num_devices=NC)

    xq = nc.declare_dram_parameter("xq", [128, KT * B], bf16, isOutput=False)
    xk = nc.declare_dram_parameter("xk", [128, KT * B], bf16, isOutput=False)
    xv = nc.declare_dram_parameter("xv", [128, KT * B], bf16, isOutput=False)
    # p-major packed weights: wq[p, kt*PC + n] = Wq[kt*128 + p, cols_c[n]]
    wq = nc.declare_dram_parameter("wq", [128, KT * PC], bf16, isOutput=False)
    wk = nc.declare_dram_parameter("wk", [128, KT * PC], bf16, isOutput=False)
    wv = nc.declare_dram_parameter("wv", [128, KT * PC], bf16, isOutput=False)
    # stacked p-major Wo: wo[p, t*PC + e] = Wo[(j=p%64)*DK + t + 32*(p//64), cols_c[e]]
    wo = nc.declare_dram_parameter("wo", [128, TT * PC], bf16, isOutput=False)
    bq = nc.declare_dram_parameter("bq", [1, PC], bf16, isOutput=False)
    bk = nc.declare_dram_parameter("bk", [1, PC], bf16, isOutput=False)
    bv = nc.declare_dram_parameter("bv", [1, PC], bf16, isOutput=False)
    bo = nc.declare_dram_parameter("bo", [1, PC], bf16, isOutput=False)
    ones = nc.declare_dram_parameter("ones", [1, B], bf16, isOutput=False)
    onescol = nc.declare_dram_parameter("onescol", [H, 1], bf16, isOutput=False)
    out = nc.declare_dram_parameter("out", [B, PC], f32, isOutput=True)

    with tile.TileContext(nc) as tc:
        with (
            tc.tile_pool(name="consts", bufs=1) as consts,
            tc.tile_pool(name="wpool", bufs=8) as wpool,
            tc.tile_pool(name="wopool", bufs=8) as wopool,
            tc.tile_pool(name="work", bufs=1) as work,
            tc.tile_pool(name="pp", bufs=3, space="PSUM") as pp,
            tc.tile_pool(name="pa", bufs=1, space="PSUM") as pa,
            tc.tile_pool(name="po", bufs=1, space="PSUM") as po,
            tc.tile_pool(name="dram", bufs=1, space="DRAM") as dram,
        ):
            # ---- constants / activations in ----
            xt = {}
            xsrc = {"v": xv, "q": xq, "k": xk}

            def load_x(name):
                t = consts.tile([128, KT * B], bf16, tag=f"x{name}")
                nc.sync.dma_start(t[:], xsrc[name][:])
                xt[name] = t

            load_x("q")
            ones_t = consts.tile([1, B], bf16, tag="ones")
            nc.sync.dma_start(ones_t[:], ones[:])
            onescol_t = consts.tile([H, 1], bf16, tag="onescol")
            nc.sync.dma_start(onescol_t[:], onescol[:])
            bias_t = {}
            for name, src in (("q", bq), ("k", bk), ("v", bv), ("o", bo)):
                t = consts.tile([1, PC], bf16, tag=f"b{name}")
                nc.sync.dma_start(t[:], src[:])
                bias_t[name] = t
            ident = consts.tile([B, B], f32, tag="ident")
            make_identity(nc, ident)

            # Warm the Exp/Sqrt activation tables off the critical path.
            warm = work.tile([1, 1], f32, tag="warm")
            nc.gpsimd.memset(warm[:], 0.0)
            nc.scalar.activation(warm[:], warm[:], AF.Exp)
            nc.scalar.activation(warm[:], warm[:], AF.Sqrt)

            ag1_in = dram.tile([HL, B], f32)                 # scores^T shard
            ag1_out = dram.tile([NC, HL, B], f32, addr_space="Shared")
            # vh payload padded past 64 KB so the runtime picks RDH, not
            # the much slower Mesh algorithm at this size
            ag2_in = dram.tile([PC + 16, B], bf16)
            ag2_out = dram.tile([NC, PC + 16, B], bf16, addr_space="Shared")

            # ---- projections (bf16, K tiled by 128); Q, K first, V last ----
            CHUNKS = [1, 1, 2] + [4] * 7

            def project(name, w, bias, after_chunk=None):
                ps = pp.tile([B, PC], f32, tag="pp")
                kt0 = 0
                for ci, chn in enumerate(CHUNKS):
                    wt = wpool.tile([128, chn * PC], bf16, tag="w")
                    nc.sync.dma_start(wt[:], w[:, kt0 * PC:(kt0 + chn) * PC])
                    if after_chunk is not None and ci == 3:
                        after_chunk()
                    for i in range(chn):
                        kt = kt0 + i
                        nc.tensor.matmul(
                            ps[:],
                            lhsT=xt[name][:, kt * B:(kt + 1) * B],
                            rhs=wt[:, i * PC:(i + 1) * PC],
                            start=(kt == 0),
                            stop=False,
                        )
                    kt0 += chn
                nc.tensor.matmul(
                    ps[:], lhsT=ones_t[:], rhs=bias_t[bias][:], start=False, stop=True
                )
                return ps

            def _prefetch_xkv():
                load_x("k")
                load_x("v")

            qh_ps = project("q", wq, "q", after_chunk=_prefetch_xkv)
            qh_s = work.tile([B, PC], f32, tag="qh_s")
            nc.vector.tensor_copy(qh_s[:], qh_ps[:])
            kh_ps = project("k", wk, "k")
            kh_s = work.tile([B, PC], f32, tag="kh_s")
            nc.scalar.activation(kh_s[:], kh_ps[:], AF.Copy)

            # ---- local cosine-sim scores (64, 8); overlaps V's DMA ----
            prod = work.tile([B, PC], f32, tag="prod")
            red = work.tile([B, 3 * HL], f32, tag="red")  # num | qq | kk
            for i, (a, b) in enumerate(((qh_s, kh_s), (qh_s, qh_s), (kh_s, kh_s))):
                nc.vector.tensor_tensor(prod[:], a[:], b[:], op=ALU.mult)
                nc.vector.reduce_sum(
                    red[:, i * HL:(i + 1) * HL].unsqueeze(-1),
                    prod[:].rearrange("p (h d) -> p h d", d=DK),
                    axis=AX.X,
                )
            den = work.tile([B, HL], f32, tag="den")
            nc.vector.tensor_tensor(
                den[:], red[:, HL:2 * HL], red[:, 2 * HL:3 * HL], op=ALU.mult
            )
            nc.scalar.activation(den[:], den[:], AF.Sqrt)
            nc.vector.tensor_scalar_max(den[:], den[:], EPS)
            nc.vector.reciprocal(den[:], den[:])
            sc = work.tile([B, HL], f32, tag="sc")
            nc.vector.tensor_tensor(sc[:], red[:, 0:HL], den[:], op=ALU.mult)
            # transpose scores (64, 8) -> (8, 64); first (cold) AllGather
            scT_ps = pp.tile([HL, B], f32, tag="pp")
            nc.tensor.transpose(scT_ps[:], sc[:], ident[:])
            scT_s = work.tile([HL, B], f32, tag="scT_s")
            nc.vector.tensor_copy(scT_s[:], scT_ps[:])
            nc.scalar.dma_start(ag1_in[:], scT_s[:])
            nc.gpsimd.collective_compute(
                "AllGather",
                ALU.bypass,
                replica_groups=[list(range(NC))],
                ins=[ag1_in.opt()],
                outs=[ag1_out.opt()],
            )

            vh_ps = project("v", wv, "v")
            vh_s = work.tile([B, PC], f32, tag="vh_s")
            nc.vector.tensor_copy(vh_s[:], vh_ps[:])
            # transpose vh (64, 512) -> vhT (512, 64) in 4 PE transposes
            vhT_ps = pp.tile([128, 4 * B], f32, tag="pp")
            for t in range(4):
                nc.tensor.transpose(
                    vhT_ps[:, t * B:(t + 1) * B], vh_s[:, t * 128:(t + 1) * 128], ident[:]
                )
            vhT_s = work.tile([128, 4 * B], bf16, tag="vhT_s")
            nc.scalar.activation(vhT_s[:], vhT_ps[:], AF.Copy)
            nc.sync.dma_start(
                ag2_in[0:PC, :].rearrange("(t p) b -> p t b", p=128),
                vhT_s[:].rearrange("p (t b) -> p t b", t=4),
            )
            # fill the 2 KB pad region (its values are never read)
            nc.scalar.dma_start(
                ag2_in[PC:PC + 16, :].bitcast(f32).rearrange("(a hl) b -> a hl b", a=2),
                scT_s[:].rearrange("hl (a b) -> a hl b", a=2),
            )
            # second (warm) AllGather: the vh head-shard
            nc.gpsimd.collective_compute(
                "AllGather",
                ALU.bypass,
                replica_groups=[list(range(NC))],
                ins=[ag2_in.opt()],
                outs=[ag2_out.opt()],
            )

            # ---- Wo chunks stream while the collectives run ----
            wo_t = []
            for ch in range(TT // WCH):
                t = wopool.tile([128, WCH * PC], bf16, tag="wo")
                nc.sync.dma_start(t[:], wo[:, ch * WCH * PC:(ch + 1) * WCH * PC])
                wo_t.append(t)

            # ---- softmax pieces: sT, exp, row sums (after AG1) ----
            sT = work.tile([H, B], f32, tag="sT")
            nc.scalar.dma_start(sT[:], ag1_out[:].rearrange("c hl b -> (c hl) b"))
            expT = work.tile([H, B], bf16, tag="expT")
            nc.scalar.activation(expT[:], sT[:], AF.Exp)
            rT_ps = pp.tile([1, B], f32, tag="pp")
            nc.tensor.matmul(rT_ps[:], lhsT=onescol_t[:], rhs=expT[:], start=True, stop=True)
            rT2 = work.tile([1, 2 * B], f32, tag="rT2")
            nc.vector.reciprocal(rT2[:, 0:B], rT_ps[:])
            nc.vector.reciprocal(rT2[:, B:2 * B], rT_ps[:])
            rinv_ps = pp.tile([128, 1], f32, tag="pp")
            nc.tensor.transpose(rinv_ps[:], rT2[:], ident[0:1, 0:1])
            rinv = work.tile([128, 1], f32, tag="rinv")
            nc.vector.tensor_copy(rinv[:], rinv_ps[:])

            # vh2[h, (d, b)] for all 64 heads, spread across 4 engine queues
            vh2 = work.tile([H, DK * B], bf16, tag="vh2")
            qeng = [nc.sync, nc.scalar]
            for c in range(NC):
                qeng[c % 2].dma_start(
                    vh2[c * HL:(c + 1) * HL].rearrange("hl (d b) -> hl d b", d=DK),
                    ag2_out[c, 0:PC, :].rearrange("(hl d) b -> hl d b", hl=HL),
                )

            # ---- einsum attn2[j, (d, b)] = expT.T @ vh2; half 1 lands in
            #      PSUM partitions 64-127, so normalization is one
            #      full-width DVE scale into bf16 attn2 ----
            HALF = DK * B // 2
            attn2 = work.tile([128, TT * B], bf16, tag="attn2")
            pah = pa.tile([128, HALF], f32, tag="pa")
            for h in range(2):
                for n in range(HALF // 512):
                    nc.tensor.matmul(
                        pah[h * B:(h + 1) * B, n * 512:(n + 1) * 512],
                        lhsT=expT[:],
                        rhs=vh2[:, h * HALF + n * 512:h * HALF + (n + 1) * 512],
                        start=True,
                        stop=True,
                    )
            nc.vector.tensor_scalar_mul(attn2[:, :], pah[:, :], rinv[:])

            # ---- out[:, cols_c] = attn @ Wo_c + bo_c : 32 K=128 matmuls ----
            op = po.tile([B, PC], f32, tag="po")
            nc.tensor.matmul(
                op[:], lhsT=ones_t[:], rhs=bias_t["o"][:], start=True, stop=False
            )
            for t in range(TT):
                nc.tensor.matmul(
                    op[:],
                    lhsT=attn2[:, t * B:(t + 1) * B],
                    rhs=wo_t[t // WCH][:, (t % WCH) * PC:(t % WCH + 1) * PC],
                    start=False,
                    stop=(t == TT - 1),
                )
            out_s = work.tile([B, PC], f32, tag="out_s")
            nc.vector.tensor_copy(out_s[:], op[:])
            nc.sync.dma_start(out[:], out_s[:])

    nc.compile()
    return nc


def _prep_inputs(q, k, v, Wq, bq, Wk, bk, Wv, bv, Wo, bo):
    import ml_dtypes

    bf16 = ml_dtypes.bfloat16

    def xt_layout(x):
        a = np.ascontiguousarray(x[:, 0, :].T)          # (L, B)
        a = a.reshape(KT, 128, B).transpose(1, 0, 2)     # (128, KT, B)
        return np.ascontiguousarray(a).reshape(128, KT * B).astype(bf16)

    def w_layout(W, cols):
        a = W[:, cols].reshape(KT, 128, PC).transpose(1, 0, 2)  # (128, KT, PC)
        return np.ascontiguousarray(a).reshape(128, KT * PC).astype(bf16)

    def wo_layout(Wo_c):
        # wo[p, t*PC + e] = Wo_c[(p % 64) * DK + t + 32 * (p // 64), e]
        a = Wo_c.reshape(B, 2, TT, PC).transpose(1, 0, 2, 3)    # (hi, j, t, e)
        return np.ascontiguousarray(a).reshape(128, TT * PC).astype(bf16)

    xq_np, xk_np, xv_np = xt_layout(q), xt_layout(k), xt_layout(v)
    ones_np = np.ones((1, B), dtype=bf16)

    in_maps = []
    for c in range(NC):
        cols = slice(c * PC, (c + 1) * PC)
        in_maps.append({
            "xq": xq_np, "xk": xk_np, "xv": xv_np,
            "wq": w_layout(Wq, cols),
            "wk": w_layout(Wk, cols),
            "wv": w_layout(Wv, cols),
            "wo": wo_layout(np.ascontiguousarray(Wo[:, cols])),
            "bq": bq[cols].reshape(1, PC).astype(bf16),
            "bk": bk[cols].reshape(1, PC).astype(bf16),
            "bv": bv[cols].reshape(1, PC).astype(bf16),
            "bo": bo[cols].reshape(1, PC).astype(bf16),
            "ones": ones_np,
            "onescol": np.ones((H, 1), dtype=bf16),
        })
    return in_maps


def _run(inputs, trace=False, **kw):
    from concourse.bass_utils import run_bass_kernel_spmd

    if "nc" not in _CACHE:
        _CACHE["nc"] = _build()
    nc = _CACHE["nc"]
    in_maps = _prep_inputs(**inputs)
    res = run_bass_kernel_spmd(nc, in_maps, list(range(NC)), trace=trace, **kw)
    full = np.concatenate([res.results[c]["out"] for c in range(NC)], axis=1)
    return full.astype(np.float32), res


def kernel(**inputs):
    out, _ = _run(inputs, trace=False)
    return out
